# revision 7
# baseline (speedup 1.0000x reference)
"""MACAttention (sparse windowed attention w/ persistent memory) on 8 TRN2 cores.

Strategy: pure data parallelism over the 16 independent (batch, window)
attention blocks -- 2 windows per core, no collectives. Per core:
  RMSNorm (folded: sumsq via ones-matmul, r[t] applied to rope tables / v)
  QKV projection (weights streamed, transposed layout, float32r matmuls)
  RoPE (even/odd head-dim permutation folded into weights host-side)
  windowed attention with 16 persistent k/v tokens (transposed layout:
  k on partitions, q on free dim; softmax without max subtraction)
  output projection.
Everything fp32 data; matmuls run as float32r (full PE rate, ~1e-4 rel).
"""
import sys

if "/opt/trn_rl_repo" not in sys.path:
    sys.path.insert(0, "/opt/trn_rl_repo")

import numpy as np
import concourse.bass as bass
from concourse import bacc
import concourse.mybir as mybir
import concourse.tile as tile
from concourse.bass_utils import run_bass_kernel_spmd

F32 = mybir.dt.float32
F32R = mybir.dt.float32r
AF = mybir.ActivationFunctionType

HEADS = 16
DH = 128
D = 2048
C = 512          # window width (q len)
NP = 16          # persistent tokens
NCORES = 8
NW = 2           # windows per core
T = NW * C       # tokens per core
DC = 16          # d-chunks (2048/128)
SCALE = DH ** -0.5
EPS = 1e-6
THETA = 10000.0

_PERM = np.concatenate([np.arange(0, DH, 2), np.arange(1, DH, 2)])  # evens|odds

# attention chunk c (1..4): cols < 128*(c-1) are fully masked; fp32r matmuls
# need a moving free dim >= 256, so the matmul col start is clamped.
_RC = [0, 0, 128, 256, 256]


def _build():
    nc = bacc.Bacc("TRN2", target_bir_lowering=False, debug=False)

    xT = nc.declare_dram_parameter("xT", [NW, 128, DC, C], F32R, isOutput=False)
    wqk = nc.declare_dram_parameter("wqk", [32, 128, DC, 128], F32R, isOutput=False)
    wv = nc.declare_dram_parameter("wv", [4, DC, 128, C], F32R, isOutput=False)
    wo = nc.declare_dram_parameter("wo", [16, 128, 16, 128], F32R, isOutput=False)
    cos_d = nc.declare_dram_parameter("cos_d", [NW, 128, C], F32, isOutput=False)
    sin_d = nc.declare_dram_parameter("sin_d", [NW, 128, C], F32, isOutput=False)
    tri_d = nc.declare_dram_parameter("tri_d", [128, 2, 128], F32, isOutput=False)
    pmk_d = nc.declare_dram_parameter("pmk_d", [128, HEADS, NP], F32R, isOutput=False)
    pmv_d = nc.declare_dram_parameter("pmv_d", [128, HEADS, DH], F32R, isOutput=False)
    ones_d = nc.declare_dram_parameter("ones_d", [128, 1], F32R, isOutput=False)
    zero_d = nc.declare_dram_parameter("zero_d", [128, C], F32R, isOutput=False)
    out = nc.declare_dram_parameter("out", [16, 128, NW, C], F32, isOutput=True)
    scratch = nc.dram_tensor("scratch", [NW, 1, C], F32)

    with tile.TileContext(nc) as tc:
        with (
            tc.tile_pool(name="stat", bufs=1) as stat,
            tc.tile_pool(name="xp", bufs=1) as xp,
            tc.tile_pool(name="wp", bufs=4) as wp,
            tc.tile_pool(name="qkp", bufs=5) as qkp,
            tc.tile_pool(name="vp", bufs=4) as vp,
            tc.tile_pool(name="aop", bufs=16) as aop,
            tc.tile_pool(name="unp", bufs=7) as unp,
            tc.tile_pool(name="tmpp", bufs=3) as tmpp,
            tc.tile_pool(name="otp", bufs=2) as otp,
            tc.tile_pool(name="rbp", bufs=2) as rbp,
            tc.tile_pool(name="tabp", bufs=1) as tabp,
            tc.tile_pool(name="smallp", bufs=2) as smallp,
            tc.tile_pool(name="ps", bufs=1, space="PSUM") as ps,
        ):
            # ---- static tiles ----
            tri = stat.tile([128, 2, 128], F32)
            nc.sync.dma_start(tri, tri_d[:, :, :])
            pmk = stat.tile([128, HEADS, NP], F32R)
            nc.sync.dma_start(pmk, pmk_d[:, :, :])
            pmv = stat.tile([128, HEADS, DH], F32R)
            nc.sync.dma_start(pmv, pmv_d[:, :, :])
            ones = stat.tile([128, 1], F32R)
            nc.sync.dma_start(ones, ones_d[:, :])
            zeros = stat.tile([128, C], F32R)
            nc.sync.dma_start(zeros, zero_d[:, :])
            zb = stat.tile([128, 1], F32)
            nc.vector.memset(zb, 0.0)
            epst = stat.tile([1, 1], F32)
            nc.vector.memset(epst, EPS)

            for w in range(NW):
                # ---- load x^T for this window ----
                xt = xp.tile([128, DC, C], F32R, tag="xt")
                nc.sync.dma_start(xt[:, 0:8, :], xT[w, :, 0:8, :])
                nc.sync.dma_start(xt[:, 8:16, :], xT[w, :, 8:16, :])

                # ---- sumsq -> r (rms scale per token) ----
                ps_sum = ps.tile([1, C], F32, tag="sum", bufs=2)
                for dc in range(DC):
                    x2 = tmpp.tile([128, C], F32R, tag="tmp")
                    nc.scalar.activation(x2, xt[:, dc, :], AF.Square, bias=zb, scale=1.0)
                    nc.tensor.matmul(ps_sum, ones, x2, start=(dc == 0), stop=(dc == DC - 1))
                sq = smallp.tile([1, C], F32, tag="sq")
                nc.scalar.activation(sq, ps_sum, AF.Sqrt, bias=epst, scale=1.0 / D)
                r_sb = smallp.tile([1, C], F32, tag="rsb")
                nc.vector.reciprocal(r_sb, sq)
                # r in token-partition layout (for v scaling), via DRAM bounce
                nc.sync.dma_start(scratch[w], r_sb[:, :])
                r_tp = smallp.tile([128, 4], F32, tag="rtp")
                with nc.allow_non_contiguous_dma(reason="tiny r transpose"):
                    nc.sync.dma_start(r_tp, scratch[w][0].rearrange("(c p) -> p c", p=128))
                # r broadcast across partitions; fold into rope tables
                rbc = tabp.tile([128, C], F32, tag="rbc")
                nc.gpsimd.partition_broadcast(rbc, r_sb[:])
                cw = tabp.tile([128, C], F32, tag="cw")
                nc.sync.dma_start(cw, cos_d[w, :, :])
                sw_ = tabp.tile([128, C], F32, tag="sw")
                nc.sync.dma_start(sw_, sin_d[w, :, :])
                cosr = tabp.tile([128, C], F32, tag="cosr")
                nc.vector.tensor_mul(cosr, cw, rbc)
                sinr = tabp.tile([128, C], F32, tag="sinr")
                nc.vector.tensor_mul(sinr, sw_, rbc)

                # ---- v pass: v[t, o] tiles [128, 2048] per t-chunk ----
                v_tiles = []
                for tch in range(4):
                    v_tiles.append(vp.tile([128, D], F32R, tag="v", name=f"v{w}_{tch}"))
                for ovb in range(4):
                    psv = [None] * 4
                    for dc in range(DC):
                        wvt = wp.tile([128, C], F32R, tag="w", name=f"wv{w}_{ovb}_{dc}")
                        nc.sync.dma_start(wvt, wv[ovb, dc, :, :])
                        for tch in range(4):
                            if dc == 0:
                                psv[tch] = ps.tile(
                                    [128, C], F32, tag="mm", bufs=4,
                                    name=f"psv{w}_{ovb}_{tch}",
                                )
                            nc.tensor.matmul(
                                psv[tch],
                                xt[:, dc, tch * 128 : (tch + 1) * 128],
                                wvt,
                                start=(dc == 0),
                                stop=(dc == DC - 1),
                            )
                    for tch in range(4):
                        nc.vector.tensor_scalar_mul(
                            v_tiles[tch][:, ovb * C : (ovb + 1) * C],
                            psv[tch],
                            r_tp[:, tch : tch + 1],
                        )

                # ---- qkv (q,k) + rope, pipelined with attention ----
                qk_tiles = [None] * 32
                ao_tiles = [None] * HEADS

                def qkv_chunk(oc):
                    wt = wp.tile([128, DC, 128], F32R, tag="w", name=f"wqk{w}_{oc}")
                    nc.sync.dma_start(wt, wqk[oc, :, :, :])
                    pq = ps.tile([128, C], F32, tag="mm", bufs=4, name=f"pq{w}_{oc}")
                    for dc in range(DC):
                        nc.tensor.matmul(
                            pq, wt[:, dc, :], xt[:, dc, :],
                            start=(dc == 0), stop=(dc == DC - 1),
                        )
                    # rope: out = pq * cosr + swap_halves(pq) * sinr
                    tmp = tmpp.tile([128, C], F32, tag="tmp", name=f"rt{w}_{oc}")
                    nc.vector.tensor_mul(tmp[0:64], pq[64:128], sinr[0:64])
                    nc.vector.tensor_mul(tmp[64:128], pq[0:64], sinr[64:128])
                    qt = qkp.tile([128, C], F32R, tag="qk", name=f"qk{w}_{oc}")
                    nc.vector.tensor_mul(qt, pq, cosr)
                    nc.vector.tensor_add(qt, qt, tmp)
                    qk_tiles[oc] = qt

                def attn_head(h):
                    q_t = qk_tiles[2 * h]
                    k_t = qk_tiles[2 * h + 1]
                    un = [None] * 5
                    # pm chunk: sim [16, C]
                    ps0 = ps.tile([16, C], F32, tag="mm", bufs=4, name=f"ps0_{w}_{h}")
                    nc.tensor.matmul(ps0, pmk[:, h, :], q_t, start=True, stop=True)
                    u0 = unp.tile([128, C], F32R, tag="un", name=f"un0_{w}_{h}")
                    nc.vector.tensor_copy(u0, zeros)
                    nc.scalar.activation(u0[0:16], ps0, AF.Exp, bias=zb[0:16], scale=SCALE)
                    un[0] = u0
                    for cch in range(1, 5):
                        cs = 128 * (cch - 1)   # diagonal block start
                        rc = _RC[cch]          # matmul col start
                        psc = ps.tile(
                            [128, C], F32, tag="mm", bufs=4, name=f"psc{w}_{h}_{cch}"
                        )
                        nc.tensor.matmul(
                            psc[:, rc:C], k_t[:, cs : cs + 128], q_t[:, rc:C],
                            start=True, stop=True,
                        )
                        uc = unp.tile([128, C], F32R, tag="un", name=f"un{w}_{h}_{cch}")
                        if cs > 0:
                            nc.vector.tensor_copy(uc[:, 0:cs], zeros[:, 0:cs])
                        nc.scalar.activation(
                            uc[:, cs:C], psc[:, cs:C], AF.Exp, bias=zb, scale=SCALE
                        )
                        nc.vector.tensor_mul(
                            uc[:, cs : cs + 128],
                            uc[:, cs : cs + 128],
                            tri[:, 1 if cch > 1 else 0, :],
                        )
                        un[cch] = uc
                    # denominators
                    ps_s = ps.tile([1, C], F32, tag="sum", bufs=2, name=f"pss{w}_{h}")
                    nc.tensor.matmul(ps_s, ones, un[0], start=True, stop=False)
                    for cch in range(1, 5):
                        rc = _RC[cch]
                        nc.tensor.matmul(
                            ps_s[:, rc:C], ones, un[cch][:, rc:C],
                            start=False, stop=(cch == 4),
                        )
                    s_sb = smallp.tile([1, C], F32, tag="ssb", name=f"ssb{w}_{h}")
                    nc.scalar.copy(s_sb, ps_s)
                    rb = rbp.tile([128, C], F32, tag="rb", name=f"rb{w}_{h}")
                    nc.gpsimd.partition_broadcast(rb, s_sb[:])
                    rbr = rbp.tile([128, C], F32, tag="rbr", name=f"rbr{w}_{h}")
                    nc.vector.reciprocal(rbr, rb)
                    # attn @ v  (out^T accumulation)
                    ps_av = ps.tile([128, C], F32, tag="av", bufs=2, name=f"pav{w}_{h}")
                    nc.tensor.matmul(ps_av, pmv[:, h, :], un[0], start=True, stop=False)
                    for cch in range(1, 5):
                        rc = _RC[cch]
                        nc.tensor.matmul(
                            ps_av[:, rc:C],
                            v_tiles[cch - 1][:, h * DH : (h + 1) * DH],
                            un[cch][:, rc:C],
                            start=False,
                            stop=(cch == 4),
                        )
                    ao = aop.tile([128, C], F32R, tag="ao", name=f"ao{w}_{h}")
                    nc.vector.tensor_mul(ao, ps_av, rbr)
                    ao_tiles[h] = ao

                # software-pipeline: emit qkv for head h, attention for head h-1
                for h in range(HEADS):
                    qkv_chunk(2 * h)
                    qkv_chunk(2 * h + 1)
                    if h >= 1:
                        attn_head(h - 1)
                attn_head(HEADS - 1)

                # ---- output projection ----
                for ec in range(16):
                    wot = wp.tile([128, 16, 128], F32R, tag="w", name=f"wo{w}_{ec}")
                    nc.sync.dma_start(wot, wo[ec, :, :, :])
                    pso = ps.tile([128, C], F32, tag="mm", bufs=4, name=f"pso{w}_{ec}")
                    for hd in range(16):
                        nc.tensor.matmul(
                            pso, wot[:, hd, :], ao_tiles[hd],
                            start=(hd == 0), stop=(hd == 15),
                        )
                    ot = otp.tile([128, C], F32, tag="ot", name=f"ot{w}_{ec}")
                    nc.scalar.copy(ot, pso)
                    nc.sync.dma_start(out[ec][:, w, :], ot)
    nc.compile()
    return nc


_NC_CACHE = None


def _get_nc():
    global _NC_CACHE
    if _NC_CACHE is None:
        _NC_CACHE = _build()
    return _NC_CACHE


def _host_prep(x, norm_w, w_qkv, w_out, pm):
    xf = np.ascontiguousarray(np.asarray(x, np.float32))
    wq = np.asarray(w_qkv, np.float32) * np.asarray(norm_w, np.float32)[None, :]
    wof = np.asarray(w_out, np.float32)
    pmf = np.asarray(pm, np.float32)

    # wqk tiles [32, 128, 16, 128]; oc=2h -> q head h, oc=2h+1 -> k head h
    wqk_heads = wq[: 2 * D].reshape(2, HEADS, DH, D)[:, :, _PERM, :]  # [s,h,dh,d]
    wqk_t = np.empty((32, 128, DC, 128), np.float32)
    for h in range(HEADS):
        for s in range(2):
            blk = wqk_heads[s, h]  # [dh(o)=128, d=2048]
            wqk_t[2 * h + s] = blk.T.reshape(DC, 128, 128).transpose(1, 0, 2)

    # wv tiles [4, 16, 128, 512]: (ovb, dc, p, o) = w_v[ovb*512+o, dc*128+p]
    wv_m = wq[2 * D :]  # [2048 ov, 2048 d]
    wv_t = np.ascontiguousarray(wv_m.reshape(4, C, DC, 128).transpose(0, 2, 3, 1))

    # wo tiles [16, 128, 16, 128]: (ec, p, hdc, e) = wo[ec*128+e, hdc*128+p]
    wo_t = np.ascontiguousarray(wof.reshape(16, 128, 16, 128).transpose(0, 3, 2, 1))

    inv = THETA ** (-np.arange(0, DH, 2, dtype=np.float64) / DH)  # [64]

    # diagonal masks [128, 2, 128]: idx0 chunk-1 (longterm rows all-valid), idx1 plain
    kr = np.arange(128)[:, None]
    qq = np.arange(128)[None, :]
    tri_plain = (qq >= kr).astype(np.float32)
    tri_c1 = tri_plain.copy()
    tri_c1[0:16, :] = 1.0
    tri_t = np.ascontiguousarray(np.stack([tri_c1, tri_plain], axis=1))

    pmk_t = np.ascontiguousarray(pmf[0][:, :, _PERM].transpose(2, 0, 1))  # [128,h,16]
    pmv_t = np.zeros((128, HEADS, DH), np.float32)
    pmv_t[0:16] = pmf[1].transpose(1, 0, 2)  # [16t, h, 128d] -> rows 0:16

    shared = {
        "wqk": wqk_t,
        "wv": wv_t,
        "wo": wo_t,
        "tri_d": tri_t,
        "pmk_d": pmk_t,
        "pmv_d": pmv_t,
        "ones_d": np.ones((128, 1), np.float32),
        "zero_d": np.zeros((128, C), np.float32),
    }

    in_maps = []
    for c in range(NCORES):
        b, tok0 = c // 4, (c % 4) * T
        xs = xf[b, tok0 : tok0 + T]  # [1024, 2048]
        xT_c = np.ascontiguousarray(xs.reshape(NW, C, DC, 128).transpose(0, 3, 2, 1))
        pos = tok0 + np.arange(T, dtype=np.float64)
        ang = pos[:, None] * inv[None, :]  # [T, 64]
        cosv = np.cos(ang).astype(np.float32).T  # [64, T]
        sinv = np.sin(ang).astype(np.float32).T
        cos_c = np.concatenate([cosv, cosv], axis=0).reshape(128, NW, C).transpose(1, 0, 2)
        sin_c = np.concatenate([-sinv, sinv], axis=0).reshape(128, NW, C).transpose(1, 0, 2)
        m = dict(shared)
        m["xT"] = xT_c
        m["cos_d"] = np.ascontiguousarray(cos_c)
        m["sin_d"] = np.ascontiguousarray(sin_c)
        in_maps.append(m)
    return in_maps


def kernel(x, norm_w, w_qkv, w_out, pm, _trace=False):
    nc = _get_nc()
    in_maps = _host_prep(x, norm_w, w_qkv, w_out, pm)
    res = run_bass_kernel_spmd(nc, in_maps, core_ids=list(range(NCORES)), trace=_trace)
    b, n = np.asarray(x).shape[0], np.asarray(x).shape[1]
    out_full = np.empty((b, n, D), np.float32)
    for c in range(NCORES):
        arr = res.results[c]["out"]  # [16, 128, NW, C]
        bb, tok0 = c // 4, (c % 4) * T
        out_full[bb, tok0 : tok0 + T] = arr.transpose(2, 3, 0, 1).reshape(T, D)
    kernel._last_results = res
    return out_full


# revision 8
# speedup vs baseline: 1.0952x; 1.0952x over previous
"""MACAttention (sparse windowed attention w/ persistent memory) on 8 TRN2 cores.

Strategy: pure data parallelism over the 16 independent (batch, window)
attention blocks -- 2 windows per core, no collectives. Per core:
  RMSNorm (folded: sumsq via ones-matmul, r[t] applied to rope tables / v)
  QKV projection (weights streamed, transposed layout, float32r matmuls)
  RoPE (even/odd head-dim permutation folded into weights host-side)
  windowed attention with 16 persistent k/v tokens (transposed layout:
  k on partitions, q on free dim; softmax without max subtraction)
  output projection.
Everything fp32 data; matmuls run as float32r (full PE rate, ~1e-4 rel).
"""
import sys

if "/opt/trn_rl_repo" not in sys.path:
    sys.path.insert(0, "/opt/trn_rl_repo")

import numpy as np
import concourse.bass as bass
from concourse import bacc
import concourse.mybir as mybir
import concourse.tile as tile
from concourse.bass_utils import run_bass_kernel_spmd

F32 = mybir.dt.float32
F32R = mybir.dt.float32r
AF = mybir.ActivationFunctionType

HEADS = 16
DH = 128
D = 2048
C = 512          # window width (q len)
NP = 16          # persistent tokens
NCORES = 8
NW = 2           # windows per core
T = NW * C       # tokens per core
DC = 16          # d-chunks (2048/128)
SCALE = DH ** -0.5
EPS = 1e-6
THETA = 10000.0

_PERM = np.concatenate([np.arange(0, DH, 2), np.arange(1, DH, 2)])  # evens|odds

# attention chunk c (1..4): cols < 128*(c-1) are fully masked; fp32r matmuls
# need a moving free dim >= 256, so the matmul col start is clamped.
_RC = [0, 0, 128, 256, 256]


def _build():
    nc = bacc.Bacc("TRN2", target_bir_lowering=False, debug=False)

    xT = nc.declare_dram_parameter("xT", [NW, 128, DC, C], F32R, isOutput=False)
    wqk = nc.declare_dram_parameter("wqk", [32, 128, DC, 128], F32R, isOutput=False)
    wv = nc.declare_dram_parameter("wv", [4, DC, 128, C], F32R, isOutput=False)
    wo = nc.declare_dram_parameter("wo", [16, 128, 16, 128], F32R, isOutput=False)
    cos_d = nc.declare_dram_parameter("cos_d", [NW, 128, C], F32, isOutput=False)
    sin_d = nc.declare_dram_parameter("sin_d", [NW, 128, C], F32, isOutput=False)
    tri_d = nc.declare_dram_parameter("tri_d", [128, 2, 128], F32, isOutput=False)
    pmk_d = nc.declare_dram_parameter("pmk_d", [128, HEADS, NP], F32R, isOutput=False)
    pmv_d = nc.declare_dram_parameter("pmv_d", [128, HEADS, DH], F32R, isOutput=False)
    ones_d = nc.declare_dram_parameter("ones_d", [128, 1], F32R, isOutput=False)
    zero_d = nc.declare_dram_parameter("zero_d", [128, C], F32R, isOutput=False)
    out = nc.declare_dram_parameter("out", [16, 128, NW, C], F32, isOutput=True)
    scratch = nc.dram_tensor("scratch", [NW, 1, C], F32)

    with tile.TileContext(nc) as tc:
        with (
            tc.tile_pool(name="stat", bufs=1) as stat,
            tc.tile_pool(name="xp", bufs=1) as xp,
            tc.tile_pool(name="wp", bufs=4) as wp,
            tc.tile_pool(name="qkp", bufs=5) as qkp,
            tc.tile_pool(name="vp", bufs=4) as vp,
            tc.tile_pool(name="aop", bufs=16) as aop,
            tc.tile_pool(name="unp", bufs=7) as unp,
            tc.tile_pool(name="tmpp", bufs=3) as tmpp,
            tc.tile_pool(name="otp", bufs=2) as otp,
            tc.tile_pool(name="rbp", bufs=2) as rbp,
            tc.tile_pool(name="tabp", bufs=1) as tabp,
            tc.tile_pool(name="smallp", bufs=2) as smallp,
            tc.tile_pool(name="ps", bufs=1, space="PSUM") as ps,
        ):
            # ---- static tiles ----
            tri = stat.tile([128, 2, 128], F32)
            nc.sync.dma_start(tri, tri_d[:, :, :])
            pmk = stat.tile([128, HEADS, NP], F32R)
            nc.sync.dma_start(pmk, pmk_d[:, :, :])
            pmv = stat.tile([128, HEADS, DH], F32R)
            nc.sync.dma_start(pmv, pmv_d[:, :, :])
            ones = stat.tile([128, 1], F32R)
            nc.sync.dma_start(ones, ones_d[:, :])
            zeros = stat.tile([128, C], F32R)
            nc.sync.dma_start(zeros, zero_d[:, :])
            zb = stat.tile([128, 1], F32)
            nc.vector.memset(zb, 0.0)
            epst = stat.tile([1, 1], F32)
            nc.vector.memset(epst, EPS)

            for w in range(NW):
                # ---- load x^T for this window ----
                xt = xp.tile([128, DC, C], F32R, tag="xt")
                nc.sync.dma_start(xt[:, 0:8, :], xT[w, :, 0:8, :])
                nc.sync.dma_start(xt[:, 8:16, :], xT[w, :, 8:16, :])

                # ---- sumsq -> r (rms scale per token) ----
                ps_sum = ps.tile([1, C], F32, tag="sum", bufs=1)
                for dc in range(DC):
                    x2 = tmpp.tile([128, C], F32R, tag="tmp")
                    nc.scalar.activation(x2, xt[:, dc, :], AF.Square, bias=zb, scale=1.0)
                    nc.tensor.matmul(ps_sum, ones, x2, start=(dc == 0), stop=(dc == DC - 1))
                sq = smallp.tile([1, C], F32, tag="sq")
                nc.scalar.activation(sq, ps_sum, AF.Sqrt, bias=epst, scale=1.0 / D)
                r_sb = smallp.tile([1, C], F32, tag="rsb")
                nc.vector.reciprocal(r_sb, sq)
                # r in token-partition layout (for v scaling), via DRAM bounce
                nc.sync.dma_start(scratch[w], r_sb[:, :])
                r_tp = smallp.tile([128, 4], F32, tag="rtp")
                with nc.allow_non_contiguous_dma(reason="tiny r transpose"):
                    nc.sync.dma_start(r_tp, scratch[w][0].rearrange("(c p) -> p c", p=128))
                # r broadcast across partitions; fold into rope tables
                rbc = tabp.tile([128, C], F32, tag="rbc")
                nc.gpsimd.partition_broadcast(rbc, r_sb[:])
                cw = tabp.tile([128, C], F32, tag="cw")
                nc.sync.dma_start(cw, cos_d[w, :, :])
                sw_ = tabp.tile([128, C], F32, tag="sw")
                nc.sync.dma_start(sw_, sin_d[w, :, :])
                cosr = tabp.tile([128, C], F32, tag="cosr")
                nc.vector.tensor_mul(cosr, cw, rbc)
                sinr = tabp.tile([128, C], F32, tag="sinr")
                nc.vector.tensor_mul(sinr, sw_, rbc)

                # ---- v pass: v[t, o] tiles [128, 2048] per t-chunk ----
                v_tiles = []
                for tch in range(4):
                    v_tiles.append(vp.tile([128, D], F32R, tag="v", name=f"v{w}_{tch}"))
                for ovb in range(4):
                    psv = [None] * 4
                    for dc in range(DC):
                        wvt = wp.tile([128, C], F32R, tag="w", name=f"wv{w}_{ovb}_{dc}")
                        nc.sync.dma_start(wvt, wv[ovb, dc, :, :])
                        for tch in range(4):
                            if dc == 0:
                                psv[tch] = ps.tile(
                                    [128, C], F32, tag="mm", bufs=5,
                                    name=f"psv{w}_{ovb}_{tch}",
                                )
                            nc.tensor.matmul(
                                psv[tch],
                                xt[:, dc, tch * 128 : (tch + 1) * 128],
                                wvt,
                                start=(dc == 0),
                                stop=(dc == DC - 1),
                            )
                    for tch in range(4):
                        nc.vector.tensor_scalar_mul(
                            v_tiles[tch][:, ovb * C : (ovb + 1) * C],
                            psv[tch],
                            r_tp[:, tch : tch + 1],
                        )

                # ---- qkv (q,k) + rope, pipelined with attention ----
                qk_tiles = [None] * 32
                ao_tiles = [None] * HEADS

                def qkv_chunk(oc):
                    wt = wp.tile([128, DC, 128], F32R, tag="w", name=f"wqk{w}_{oc}")
                    nc.sync.dma_start(wt, wqk[oc, :, :, :])
                    pq = ps.tile([128, C], F32, tag="mm", bufs=5, name=f"pq{w}_{oc}")
                    for dc in range(DC):
                        nc.tensor.matmul(
                            pq, wt[:, dc, :], xt[:, dc, :],
                            start=(dc == 0), stop=(dc == DC - 1),
                        )
                    # rope: out = pq * cosr + swap_halves(pq) * sinr
                    tmp = tmpp.tile([128, C], F32, tag="tmp", name=f"rt{w}_{oc}")
                    nc.vector.tensor_mul(tmp[0:64], pq[64:128], sinr[0:64])
                    nc.vector.tensor_mul(tmp[64:128], pq[0:64], sinr[64:128])
                    qt = qkp.tile([128, C], F32R, tag="qk", name=f"qk{w}_{oc}")
                    nc.vector.tensor_mul(qt, pq, cosr)
                    nc.vector.tensor_add(qt, qt, tmp)
                    qk_tiles[oc] = qt

                def attn_head(h):
                    q_t = qk_tiles[2 * h]
                    k_t = qk_tiles[2 * h + 1]
                    un = [None] * 5
                    # pm chunk: sim [16, C]
                    ps0 = ps.tile([16, C], F32, tag="mm", bufs=5, name=f"ps0_{w}_{h}")
                    nc.tensor.matmul(ps0, pmk[:, h, :], q_t, start=True, stop=True)
                    u0 = unp.tile([128, C], F32R, tag="un", name=f"un0_{w}_{h}")
                    nc.vector.tensor_copy(u0, zeros)
                    nc.scalar.activation(u0[0:16], ps0, AF.Exp, bias=zb[0:16], scale=SCALE)
                    un[0] = u0
                    for cch in range(1, 5):
                        cs = 128 * (cch - 1)   # diagonal block start
                        rc = _RC[cch]          # matmul col start
                        psc = ps.tile(
                            [128, C], F32, tag="mm", bufs=5, name=f"psc{w}_{h}_{cch}"
                        )
                        nc.tensor.matmul(
                            psc[:, rc:C], k_t[:, cs : cs + 128], q_t[:, rc:C],
                            start=True, stop=True,
                        )
                        uc = unp.tile([128, C], F32R, tag="un", name=f"un{w}_{h}_{cch}")
                        if cs > 0:
                            nc.vector.tensor_copy(uc[:, 0:cs], zeros[:, 0:cs])
                        nc.scalar.activation(
                            uc[:, cs:C], psc[:, cs:C], AF.Exp, bias=zb, scale=SCALE
                        )
                        nc.vector.tensor_mul(
                            uc[:, cs : cs + 128],
                            uc[:, cs : cs + 128],
                            tri[:, 1 if cch > 1 else 0, :],
                        )
                        un[cch] = uc
                    # denominators
                    ps_s = ps.tile([1, C], F32, tag="sum", bufs=1, name=f"pss{w}_{h}")
                    nc.tensor.matmul(ps_s, ones, un[0], start=True, stop=False)
                    for cch in range(1, 5):
                        rc = _RC[cch]
                        nc.tensor.matmul(
                            ps_s[:, rc:C], ones, un[cch][:, rc:C],
                            start=False, stop=(cch == 4),
                        )
                    lnt = smallp.tile([1, C], F32, tag="lnt", name=f"lnt{w}_{h}")
                    nc.scalar.activation(lnt, ps_s, AF.Ln, bias=zb[0:1], scale=1.0)
                    rcp = smallp.tile([1, C], F32, tag="rcp", name=f"rcp{w}_{h}")
                    nc.scalar.activation(rcp, lnt, AF.Exp, bias=zb[0:1], scale=-1.0)
                    rb = rbp.tile([128, C], F32, tag="rb", name=f"rb{w}_{h}")
                    nc.gpsimd.partition_broadcast(rb, rcp[:])
                    # attn @ v  (out^T accumulation)
                    ps_av = ps.tile([128, C], F32, tag="av", bufs=2, name=f"pav{w}_{h}")
                    nc.tensor.matmul(ps_av, pmv[:, h, :], un[0], start=True, stop=False)
                    for cch in range(1, 5):
                        rc = _RC[cch]
                        nc.tensor.matmul(
                            ps_av[:, rc:C],
                            v_tiles[cch - 1][:, h * DH : (h + 1) * DH],
                            un[cch][:, rc:C],
                            start=False,
                            stop=(cch == 4),
                        )
                    ao = aop.tile([128, C], F32R, tag="ao", name=f"ao{w}_{h}")
                    nc.vector.tensor_mul(ao, ps_av, rb)
                    ao_tiles[h] = ao

                # software-pipeline: emit qkv for head h, attention for head h-1
                for h in range(HEADS):
                    qkv_chunk(2 * h)
                    qkv_chunk(2 * h + 1)
                    if h >= 1:
                        attn_head(h - 1)
                attn_head(HEADS - 1)

                # ---- output projection ----
                for ec in range(16):
                    wot = wp.tile([128, 16, 128], F32R, tag="w", name=f"wo{w}_{ec}")
                    nc.sync.dma_start(wot, wo[ec, :, :, :])
                    pso = ps.tile([128, C], F32, tag="mm", bufs=5, name=f"pso{w}_{ec}")
                    for hd in range(16):
                        nc.tensor.matmul(
                            pso, wot[:, hd, :], ao_tiles[hd],
                            start=(hd == 0), stop=(hd == 15),
                        )
                    ot = otp.tile([128, C], F32, tag="ot", name=f"ot{w}_{ec}")
                    nc.scalar.copy(ot, pso)
                    nc.sync.dma_start(out[ec][:, w, :], ot)
    nc.compile()
    return nc


_NC_CACHE = None


def _get_nc():
    global _NC_CACHE
    if _NC_CACHE is None:
        _NC_CACHE = _build()
    return _NC_CACHE


def _host_prep(x, norm_w, w_qkv, w_out, pm):
    xf = np.ascontiguousarray(np.asarray(x, np.float32))
    wq = np.asarray(w_qkv, np.float32) * np.asarray(norm_w, np.float32)[None, :]
    wof = np.asarray(w_out, np.float32)
    pmf = np.asarray(pm, np.float32)

    # wqk tiles [32, 128, 16, 128]; oc=2h -> q head h, oc=2h+1 -> k head h
    wqk_heads = wq[: 2 * D].reshape(2, HEADS, DH, D)[:, :, _PERM, :]  # [s,h,dh,d]
    wqk_t = np.empty((32, 128, DC, 128), np.float32)
    for h in range(HEADS):
        for s in range(2):
            blk = wqk_heads[s, h]  # [dh(o)=128, d=2048]
            wqk_t[2 * h + s] = blk.T.reshape(DC, 128, 128).transpose(1, 0, 2)

    # wv tiles [4, 16, 128, 512]: (ovb, dc, p, o) = w_v[ovb*512+o, dc*128+p]
    wv_m = wq[2 * D :]  # [2048 ov, 2048 d]
    wv_t = np.ascontiguousarray(wv_m.reshape(4, C, DC, 128).transpose(0, 2, 3, 1))

    # wo tiles [16, 128, 16, 128]: (ec, p, hdc, e) = wo[ec*128+e, hdc*128+p]
    wo_t = np.ascontiguousarray(wof.reshape(16, 128, 16, 128).transpose(0, 3, 2, 1))

    inv = THETA ** (-np.arange(0, DH, 2, dtype=np.float64) / DH)  # [64]

    # diagonal masks [128, 2, 128]: idx0 chunk-1 (longterm rows all-valid), idx1 plain
    kr = np.arange(128)[:, None]
    qq = np.arange(128)[None, :]
    tri_plain = (qq >= kr).astype(np.float32)
    tri_c1 = tri_plain.copy()
    tri_c1[0:16, :] = 1.0
    tri_t = np.ascontiguousarray(np.stack([tri_c1, tri_plain], axis=1))

    pmk_t = np.ascontiguousarray(pmf[0][:, :, _PERM].transpose(2, 0, 1))  # [128,h,16]
    pmv_t = np.zeros((128, HEADS, DH), np.float32)
    pmv_t[0:16] = pmf[1].transpose(1, 0, 2)  # [16t, h, 128d] -> rows 0:16

    shared = {
        "wqk": wqk_t,
        "wv": wv_t,
        "wo": wo_t,
        "tri_d": tri_t,
        "pmk_d": pmk_t,
        "pmv_d": pmv_t,
        "ones_d": np.ones((128, 1), np.float32),
        "zero_d": np.zeros((128, C), np.float32),
    }

    in_maps = []
    for c in range(NCORES):
        b, tok0 = c // 4, (c % 4) * T
        xs = xf[b, tok0 : tok0 + T]  # [1024, 2048]
        xT_c = np.ascontiguousarray(xs.reshape(NW, C, DC, 128).transpose(0, 3, 2, 1))
        pos = tok0 + np.arange(T, dtype=np.float64)
        ang = pos[:, None] * inv[None, :]  # [T, 64]
        cosv = np.cos(ang).astype(np.float32).T  # [64, T]
        sinv = np.sin(ang).astype(np.float32).T
        cos_c = np.concatenate([cosv, cosv], axis=0).reshape(128, NW, C).transpose(1, 0, 2)
        sin_c = np.concatenate([-sinv, sinv], axis=0).reshape(128, NW, C).transpose(1, 0, 2)
        m = dict(shared)
        m["xT"] = xT_c
        m["cos_d"] = np.ascontiguousarray(cos_c)
        m["sin_d"] = np.ascontiguousarray(sin_c)
        in_maps.append(m)
    return in_maps


def kernel(x, norm_w, w_qkv, w_out, pm, _trace=False):
    nc = _get_nc()
    in_maps = _host_prep(x, norm_w, w_qkv, w_out, pm)
    res = run_bass_kernel_spmd(nc, in_maps, core_ids=list(range(NCORES)), trace=_trace)
    b, n = np.asarray(x).shape[0], np.asarray(x).shape[1]
    out_full = np.empty((b, n, D), np.float32)
    for c in range(NCORES):
        arr = res.results[c]["out"]  # [16, 128, NW, C]
        bb, tok0 = c // 4, (c % 4) * T
        out_full[bb, tok0 : tok0 + T] = arr.transpose(2, 3, 0, 1).reshape(T, D)
    kernel._last_results = res
    return out_full
